# revision 20
# baseline (speedup 1.0000x reference)
"""BitLinear (LayerNorm + sign-quantized linear) Trainium2 kernel.

Computes, for x:(8,2048,2048) f32, weight:(2048,2048) f32:
    xn  = LayerNorm(x, eps=1e-5) * ln_weight + ln_bias
    out = (xn @ sign(weight).T) * max|weight| * gamma + beta

Sharding: data-parallel over the batch dim — core c processes x[c]
(2048 rows), with a full copy of the weight. No collectives.

Queue/engine layout (per core), chosen so no instruction stream ever
waits on a resource another stream needs soon:
  GpSimd/SWDGE : all HBM loads (x, w), scale roundtrip, out stores
  Sync ring    : weight-transpose XBAR DMAs only
  ACT          : sign, sqrt, normalize, xs-transpose XBAR (right after
                 the normalize that feeds it), PSUM epilogue w/ scale
  DVE          : bn_stats/bn_aggr, |w| max reduces, reciprocal
  PE           : 1024 bf16 matmuls, gated on wT fully resident so the
                 stream is dense and the HAM clock-gate stays warm
"""

import os

import numpy as np

import concourse.bacc as bacc
import concourse.tile as tile
from concourse import mybir
from concourse.masks import make_identity
from concourse.tile_rust import add_dep_helper
from concourse.bass_utils import run_bass_kernel_spmd

P = 128
R = 2048          # rows per core (seq len)
K = 2048          # d_in
O = 2048          # d_out
B = 8             # batch == n_cores
EPS = 1e-5
RT, KT, OT = R // P, K // P, O // P
NSUB = K // 512   # bn_stats subgroups
OC = 512          # psum chunk (one f32 bank)
NOC = O // OC

F32 = mybir.dt.float32
BF16 = mybir.dt.bfloat16

PRE = 4           # LayerNorm chains emitted before weight prep
LA = 2            # matmul lookahead behind LayerNorm emission


def _build(trivial_ln_w, trivial_ln_b, trivial_gamma, trivial_beta):
    nc = bacc.Bacc("TRN2", target_bir_lowering=False, debug=False)
    x = nc.dram_tensor("x", [R, K], F32, kind="ExternalInput").ap()
    w = nc.dram_tensor("w", [O, K], F32, kind="ExternalInput").ap()
    out = nc.dram_tensor("out", [R, O], F32, kind="ExternalOutput").ap()
    scr128 = nc.dram_tensor("scr128", [P], F32).ap()
    scr1 = nc.dram_tensor("scr1", [1], F32).ap()
    need_vecs = not (trivial_ln_w and trivial_ln_b and trivial_gamma
                     and trivial_beta)
    if need_vecs:
        lnw_d = nc.dram_tensor("ln_weight", [K], F32, kind="ExternalInput").ap()
        lnb_d = nc.dram_tensor("ln_bias", [K], F32, kind="ExternalInput").ap()
        gam_d = nc.dram_tensor("gamma", [O], F32, kind="ExternalInput").ap()
        bet_d = nc.dram_tensor("beta", [O], F32, kind="ExternalInput").ap()

    with tile.TileContext(nc) as tc:
        with (
            tc.tile_pool(name="singles", bufs=1) as singles,
            tc.tile_pool(name="wload", bufs=4) as wload,
            tc.tile_pool(name="sgn", bufs=3) as sgnp,
            tc.tile_pool(name="xload", bufs=4) as xload,
            tc.tile_pool(name="xs", bufs=2) as xsp,
            tc.tile_pool(name="xsT", bufs=3) as xsTp,
            tc.tile_pool(name="stats", bufs=8) as statsp,
            tc.tile_pool(name="outp", bufs=2) as outp,
            tc.tile_pool(name="psum", bufs=4, space="PSUM") as psump,
            tc.tile_pool(name="wpsum", bufs=2, space="PSUM") as wpsump,
        ):
            wT = singles.tile([P, KT, O], BF16)
            wmax_parts = singles.tile([P, OT], F32)
            eps_t = singles.tile([P, 1], F32)
            nc.vector.memset(eps_t, EPS)
            ident = singles.tile([P, P], F32)
            make_identity(nc, ident)
            if need_vecs:
                lnw_b = singles.tile([P, K], F32)
                nc.gpsimd.dma_start(out=lnw_b, in_=lnw_d.to_broadcast([P, K]))
                lnb_b = singles.tile([P, K], F32)
                nc.gpsimd.dma_start(out=lnb_b, in_=lnb_d.to_broadcast([P, K]))
                gam_b = singles.tile([P, O], F32)
                nc.gpsimd.dma_start(out=gam_b, in_=gam_d.to_broadcast([P, O]))
                bet_b = singles.tile([P, O], F32)
                nc.gpsimd.dma_start(out=bet_b, in_=bet_d.to_broadcast([P, O]))

            xsT_tiles = {}
            xs_pair = {}
            last_wload = None

            def emit_ln(mi):
                x_t = xload.tile([P, K], F32)
                nc.sync.dma_start(out=x_t, in_=x[mi * P:(mi + 1) * P, :])
                st = statsp.tile([P, NSUB, 6], F32)
                for sgi in range(NSUB):
                    nc.vector.bn_stats(
                        out=st[:, sgi, :],
                        in_=x_t[:, sgi * 512:(sgi + 1) * 512])
                mv = statsp.tile([P, 2], F32)
                nc.vector.bn_aggr(out=mv, in_=st)
                std = statsp.tile([P, 1], F32)
                nc.scalar.activation(
                    out=std, in_=mv[:, 1:2],
                    func=mybir.ActivationFunctionType.Sqrt, bias=eps_t)
                rstd = statsp.tile([P, 1], F32)
                nc.vector.reciprocal(out=rstd, in_=std)
                if mi % 2 == 0:
                    xs_pair[mi // 2] = xsp.tile([P, 2, K], BF16, name="xs_pair", tag="xs_pair")
                xs = xs_pair[mi // 2][:, mi % 2, :]
                if trivial_ln_w and trivial_ln_b:
                    nc.vector.tensor_scalar(
                        out=xs, in0=x_t, scalar1=mv[:, 0:1], scalar2=rstd,
                        op0=mybir.AluOpType.subtract,
                        op1=mybir.AluOpType.mult)
                else:
                    xn = xsp.tile([P, K], F32, tag="xn_f32")
                    nc.vector.tensor_scalar(
                        out=xn, in0=x_t, scalar1=mv[:, 0:1], scalar2=rstd,
                        op0=mybir.AluOpType.subtract,
                        op1=mybir.AluOpType.mult)
                    if not trivial_ln_w:
                        nc.vector.tensor_mul(out=xn, in0=xn, in1=lnw_b)
                    if not trivial_ln_b:
                        nc.vector.tensor_add(out=xn, in0=xn, in1=lnb_b)
                    nc.vector.tensor_copy(out=xs, in_=xn)
                if mi % 2 == 1:
                    # one XBAR transpose covers the whole pair:
                    # [128, 2*K] -> [128, 2*KT, 128] (f = c*128 + kp)
                    xsT = xsTp.tile([P, 2 * KT, P], BF16)
                    nc.sync.dma_start_transpose(
                        out=xsT, in_=xs_pair.pop(mi // 2).rearrange(
                            "p a k -> p (a k)"))
                    xsT_tiles[mi // 2] = xsT

            # LN pipeline head: fills DVE/ACT while weights stream in
            for mi in range(PRE):
                emit_ln(mi)

            # ---------------- weight prep ----------------
            # PE transposes each 128x128 f32 chunk into PSUM; ACT applies
            # sign while evacuating PSUM -> wT bf16.
            HK = 8  # k-chunks per psum evac tile (2 banks)
            for oi in range(OT):
                w_t = wload.tile([P, K], F32)
                last_wload = nc.gpsimd.dma_start(
                    out=w_t, in_=w[oi * P:(oi + 1) * P, :])
                nc.vector.tensor_reduce(
                    out=wmax_parts[:, oi:oi + 1], in_=w_t,
                    axis=mybir.AxisListType.X, op=mybir.AluOpType.max,
                    apply_absolute_value=True)
                for h in range(KT // HK):
                    ps = wpsump.tile([P, HK, P], F32)
                    for j in range(HK):
                        kt = h * HK + j
                        last_wxp = nc.tensor.transpose(
                            ps[:, j, :], w_t[:, kt * P:(kt + 1) * P], ident)
                    nc.scalar.sign(
                        out=wT[:, h * HK:(h + 1) * HK,
                               oi * P:(oi + 1) * P],
                        in_=ps)
            # scale = cross-partition max of per-partition maxima
            wmax = singles.tile([P, 1], F32)
            nc.vector.tensor_reduce(
                out=wmax, in_=wmax_parts, axis=mybir.AxisListType.X,
                op=mybir.AluOpType.max)
            nc.gpsimd.dma_start(out=scr128[:], in_=wmax)
            srow = singles.tile([1, P], F32)
            nc.gpsimd.dma_start(
                out=srow, in_=scr128.rearrange("(a p) -> a p", a=1))
            sred = singles.tile([1, 1], F32)
            nc.vector.tensor_reduce(
                out=sred, in_=srow, axis=mybir.AxisListType.X,
                op=mybir.AluOpType.max)
            nc.gpsimd.dma_start(out=scr1[:], in_=sred)
            scale_all = singles.tile([P, 1], F32)
            nc.gpsimd.dma_start(out=scale_all, in_=scr1.to_broadcast([P, 1]))

            # ---------------- main (software-pipelined emission) --------
            first_mm = [None]

            def emit_mm(mi):
                xsT = xsT_tiles[mi // 2]
                jb = (mi % 2) * KT
                o_t = outp.tile([P, O], F32)
                for oc in range(NOC):
                    ps = psump.tile([P, OC], F32)
                    for kt in range(KT):
                        mmh = nc.tensor.matmul(
                            ps, lhsT=xsT[:, jb + kt, :],
                            rhs=wT[:, kt, oc * OC:(oc + 1) * OC],
                            start=(kt == 0), stop=(kt == KT - 1))
                        if first_mm[0] is None:
                            first_mm[0] = mmh
                            add_dep_helper(
                                mmh.ins, last_wxp.ins, sync=False,
                                reason="real MMs after weight transposes")
                    osl = o_t[:, oc * OC:(oc + 1) * OC]
                    if trivial_gamma:
                        nc.scalar.activation(
                            out=osl, in_=ps,
                            func=mybir.ActivationFunctionType.Copy,
                            scale=scale_all)
                    else:
                        gsl = gam_b[:, oc * OC:(oc + 1) * OC]
                        nc.vector.tensor_scalar_mul(
                            out=osl, in0=ps, scalar1=scale_all)
                        nc.vector.tensor_mul(out=osl, in0=osl, in1=gsl)
                    if not trivial_beta:
                        nc.vector.tensor_add(
                            out=osl, in0=osl,
                            in1=bet_b[:, oc * OC:(oc + 1) * OC])
                nc.scalar.dma_start(
                    out=out[mi * P:(mi + 1) * P, :], in_=o_t)

            for step in range(RT + LA):
                if PRE <= step < RT:
                    emit_ln(step)
                if step >= LA:
                    emit_mm(step - LA)
    nc.compile()
    return nc


def kernel(**inputs: np.ndarray) -> np.ndarray:
    x = np.asarray(inputs["x"], dtype=np.float32)
    weight = np.asarray(inputs["weight"], dtype=np.float32)
    gamma = np.asarray(inputs["gamma"], dtype=np.float32)
    beta = np.asarray(inputs["beta"], dtype=np.float32)
    ln_weight = np.asarray(inputs["ln_weight"], dtype=np.float32)
    ln_bias = np.asarray(inputs["ln_bias"], dtype=np.float32)
    assert x.shape == (B, R, K), x.shape
    assert weight.shape == (O, K), weight.shape

    triv_lnw = bool(np.all(ln_weight == 1.0))
    triv_lnb = bool(np.all(ln_bias == 0.0))
    triv_gam = bool(np.all(gamma == 1.0))
    triv_bet = bool(np.all(beta == 0.0))
    nc = _build(triv_lnw, triv_lnb, triv_gam, triv_bet)

    in_maps = []
    for c in range(B):
        m = {"x": np.ascontiguousarray(x[c]), "w": weight}
        if not (triv_lnw and triv_lnb and triv_gam and triv_bet):
            m.update({"ln_weight": ln_weight, "ln_bias": ln_bias,
                      "gamma": gamma, "beta": beta})
        in_maps.append(m)

    trace = bool(os.environ.get("BITLIN_TRACE"))
    kwargs = {}
    if trace:
        import concourse.bass_utils as bu
        bu.upload_artifacts = lambda d: d  # keep artifacts local
        tdir = os.environ.get("BITLIN_TRACE_DIR") or None
        kwargs = {"trace": True, "tmpdir": tdir}

    res = run_bass_kernel_spmd(nc, in_maps, core_ids=list(range(B)), **kwargs)
    if trace:
        print(f"HW exec time: {res.exec_time_ns} ns")
    return np.stack([r["out"] for r in res.results], axis=0)


# revision 21
# speedup vs baseline: 1.0980x; 1.0980x over previous
"""BitLinear (LayerNorm + sign-quantized linear) Trainium2 kernel.

Computes, for x:(8,2048,2048) f32, weight:(2048,2048) f32:
    xn  = LayerNorm(x, eps=1e-5) * ln_weight + ln_bias
    out = (xn @ sign(weight).T) * max|weight| * gamma + beta

Sharding: data-parallel over the batch dim — core c processes x[c]
(2048 rows), with a full copy of the weight. No collectives.

Queue/engine layout (per core), chosen so no instruction stream ever
waits on a resource another stream needs soon:
  GpSimd/SWDGE : all HBM loads (x, w), scale roundtrip, out stores
  Sync ring    : weight-transpose XBAR DMAs only
  ACT          : sign, sqrt, normalize, xs-transpose XBAR (right after
                 the normalize that feeds it), PSUM epilogue w/ scale
  DVE          : bn_stats/bn_aggr, |w| max reduces, reciprocal
  PE           : 1024 bf16 matmuls, gated on wT fully resident so the
                 stream is dense and the HAM clock-gate stays warm
"""

import os

import numpy as np

import concourse.bacc as bacc
import concourse.tile as tile
from concourse import mybir
from concourse.masks import make_identity
from concourse.tile_rust import add_dep_helper
from concourse.bass_utils import run_bass_kernel_spmd

P = 128
R = 2048          # rows per core (seq len)
K = 2048          # d_in
O = 2048          # d_out
B = 8             # batch == n_cores
EPS = 1e-5
RT, KT, OT = R // P, K // P, O // P
NSUB = K // 512   # bn_stats subgroups
OC = 512          # psum chunk (one f32 bank)
NOC = O // OC

F32 = mybir.dt.float32
BF16 = mybir.dt.bfloat16

PRE = 2           # LayerNorm chains emitted before weight prep
LA = 2            # matmul lookahead behind LayerNorm emission


def _build(trivial_ln_w, trivial_ln_b, trivial_gamma, trivial_beta):
    nc = bacc.Bacc("TRN2", target_bir_lowering=False, debug=False)
    x = nc.dram_tensor("x", [R, K], F32, kind="ExternalInput").ap()
    w = nc.dram_tensor("w", [O, K], F32, kind="ExternalInput").ap()
    out = nc.dram_tensor("out", [R, O], F32, kind="ExternalOutput").ap()
    scr128 = nc.dram_tensor("scr128", [P], F32).ap()
    scr1 = nc.dram_tensor("scr1", [1], F32).ap()
    need_vecs = not (trivial_ln_w and trivial_ln_b and trivial_gamma
                     and trivial_beta)
    if need_vecs:
        lnw_d = nc.dram_tensor("ln_weight", [K], F32, kind="ExternalInput").ap()
        lnb_d = nc.dram_tensor("ln_bias", [K], F32, kind="ExternalInput").ap()
        gam_d = nc.dram_tensor("gamma", [O], F32, kind="ExternalInput").ap()
        bet_d = nc.dram_tensor("beta", [O], F32, kind="ExternalInput").ap()

    with tile.TileContext(nc) as tc:
        with (
            tc.tile_pool(name="singles", bufs=1) as singles,
            tc.tile_pool(name="wload", bufs=4) as wload,
            tc.tile_pool(name="sgn", bufs=3) as sgnp,
            tc.tile_pool(name="xload", bufs=3) as xload,
            tc.tile_pool(name="xs", bufs=2) as xsp,
            tc.tile_pool(name="xsT", bufs=3) as xsTp,
            tc.tile_pool(name="stats", bufs=8) as statsp,
            tc.tile_pool(name="outp", bufs=2) as outp,
            tc.tile_pool(name="psum", bufs=4, space="PSUM") as psump,
            tc.tile_pool(name="wpsum", bufs=2, space="PSUM") as wpsump,
        ):
            wT = singles.tile([P, KT, O], BF16)
            wmax_parts = singles.tile([P, OT], F32)
            eps_t = singles.tile([P, 1], F32)
            nc.vector.memset(eps_t, EPS)
            ident = singles.tile([P, P], F32)
            make_identity(nc, ident)
            if need_vecs:
                lnw_b = singles.tile([P, K], F32)
                nc.gpsimd.dma_start(out=lnw_b, in_=lnw_d.to_broadcast([P, K]))
                lnb_b = singles.tile([P, K], F32)
                nc.gpsimd.dma_start(out=lnb_b, in_=lnb_d.to_broadcast([P, K]))
                gam_b = singles.tile([P, O], F32)
                nc.gpsimd.dma_start(out=gam_b, in_=gam_d.to_broadcast([P, O]))
                bet_b = singles.tile([P, O], F32)
                nc.gpsimd.dma_start(out=bet_b, in_=bet_d.to_broadcast([P, O]))

            xsT_tiles = {}
            xs_pair = {}
            last_wload = None

            def emit_ln(mi):
                x_t = xload.tile([P, K], F32)
                nc.sync.dma_start(out=x_t, in_=x[mi * P:(mi + 1) * P, :])
                st = statsp.tile([P, NSUB, 6], F32)
                for sgi in range(NSUB):
                    nc.vector.bn_stats(
                        out=st[:, sgi, :],
                        in_=x_t[:, sgi * 512:(sgi + 1) * 512])
                mv = statsp.tile([P, 2], F32)
                nc.vector.bn_aggr(out=mv, in_=st)
                std = statsp.tile([P, 1], F32)
                nc.scalar.activation(
                    out=std, in_=mv[:, 1:2],
                    func=mybir.ActivationFunctionType.Sqrt, bias=eps_t)
                rstd = statsp.tile([P, 1], F32)
                nc.vector.reciprocal(out=rstd, in_=std)
                if mi % 2 == 0:
                    xs_pair[mi // 2] = xsp.tile([P, 2, K], BF16, name="xs_pair", tag="xs_pair")
                xs = xs_pair[mi // 2][:, mi % 2, :]
                if trivial_ln_w and trivial_ln_b:
                    if 2 <= mi < 6:
                        # handoff-critical rows: normalize on ACT so the
                        # DVE's prep-time |w| reduces can't delay them
                        negmr = statsp.tile([P, 1], F32)
                        nc.vector.tensor_scalar(
                            out=negmr, in0=mv[:, 0:1], scalar1=rstd,
                            scalar2=-1.0, op0=mybir.AluOpType.mult,
                            op1=mybir.AluOpType.mult)
                        nc.scalar.activation(
                            out=xs, in_=x_t,
                            func=mybir.ActivationFunctionType.Identity,
                            bias=negmr, scale=rstd)
                    else:
                        nc.vector.tensor_scalar(
                            out=xs, in0=x_t, scalar1=mv[:, 0:1],
                            scalar2=rstd, op0=mybir.AluOpType.subtract,
                            op1=mybir.AluOpType.mult)
                else:
                    xn = xsp.tile([P, K], F32, tag="xn_f32")
                    nc.vector.tensor_scalar(
                        out=xn, in0=x_t, scalar1=mv[:, 0:1], scalar2=rstd,
                        op0=mybir.AluOpType.subtract,
                        op1=mybir.AluOpType.mult)
                    if not trivial_ln_w:
                        nc.vector.tensor_mul(out=xn, in0=xn, in1=lnw_b)
                    if not trivial_ln_b:
                        nc.vector.tensor_add(out=xn, in0=xn, in1=lnb_b)
                    nc.vector.tensor_copy(out=xs, in_=xn)
                if mi % 2 == 1:
                    # one XBAR transpose covers the whole pair:
                    # [128, 2*K] -> [128, 2*KT, 128] (f = c*128 + kp)
                    xsT = xsTp.tile([P, 2 * KT, P], BF16)
                    nc.sync.dma_start_transpose(
                        out=xsT, in_=xs_pair.pop(mi // 2).rearrange(
                            "p a k -> p (a k)"))
                    xsT_tiles[mi // 2] = xsT

            # LN pipeline head: fills DVE/ACT while weights stream in
            for mi in range(PRE):
                emit_ln(mi)

            # ---------------- weight prep ----------------
            # PE transposes each 128x128 f32 chunk into PSUM; ACT applies
            # sign while evacuating PSUM -> wT bf16.
            HK = 8  # k-chunks per psum evac tile (2 banks)
            for oi in range(OT):
                w_t = wload.tile([P, K], F32)
                last_wload = nc.gpsimd.dma_start(
                    out=w_t, in_=w[oi * P:(oi + 1) * P, :])
                nc.vector.tensor_reduce(
                    out=wmax_parts[:, oi:oi + 1], in_=w_t,
                    axis=mybir.AxisListType.X, op=mybir.AluOpType.max,
                    apply_absolute_value=True)
                for h in range(KT // HK):
                    ps = wpsump.tile([P, HK, P], F32)
                    for j in range(HK):
                        kt = h * HK + j
                        last_wxp = nc.tensor.transpose(
                            ps[:, j, :], w_t[:, kt * P:(kt + 1) * P], ident)
                    nc.scalar.sign(
                        out=wT[:, h * HK:(h + 1) * HK,
                               oi * P:(oi + 1) * P],
                        in_=ps)
            # scale = cross-partition max of per-partition maxima
            wmax = singles.tile([P, 1], F32)
            nc.vector.tensor_reduce(
                out=wmax, in_=wmax_parts, axis=mybir.AxisListType.X,
                op=mybir.AluOpType.max)
            nc.gpsimd.dma_start(out=scr128[:], in_=wmax)
            srow = singles.tile([1, P], F32)
            nc.gpsimd.dma_start(
                out=srow, in_=scr128.rearrange("(a p) -> a p", a=1))
            sred = singles.tile([1, 1], F32)
            nc.vector.tensor_reduce(
                out=sred, in_=srow, axis=mybir.AxisListType.X,
                op=mybir.AluOpType.max)
            nc.gpsimd.dma_start(out=scr1[:], in_=sred)
            scale_all = singles.tile([P, 1], F32)
            nc.gpsimd.dma_start(out=scale_all, in_=scr1.to_broadcast([P, 1]))

            # ---------------- main (software-pipelined emission) --------
            first_mm = [None]

            def emit_mm(mi):
                xsT = xsT_tiles[mi // 2]
                jb = (mi % 2) * KT
                o_t = outp.tile([P, O], F32)
                for oc in range(NOC):
                    ps = psump.tile([P, OC], F32)
                    for kt in range(KT):
                        mmh = nc.tensor.matmul(
                            ps, lhsT=xsT[:, jb + kt, :],
                            rhs=wT[:, kt, oc * OC:(oc + 1) * OC],
                            start=(kt == 0), stop=(kt == KT - 1))
                        if first_mm[0] is None:
                            first_mm[0] = mmh
                            add_dep_helper(
                                mmh.ins, last_wxp.ins, sync=False,
                                reason="real MMs after weight transposes")
                    osl = o_t[:, oc * OC:(oc + 1) * OC]
                    if trivial_gamma:
                        nc.scalar.activation(
                            out=osl, in_=ps,
                            func=mybir.ActivationFunctionType.Copy,
                            scale=scale_all)
                    else:
                        gsl = gam_b[:, oc * OC:(oc + 1) * OC]
                        nc.vector.tensor_scalar_mul(
                            out=osl, in0=ps, scalar1=scale_all)
                        nc.vector.tensor_mul(out=osl, in0=osl, in1=gsl)
                    if not trivial_beta:
                        nc.vector.tensor_add(
                            out=osl, in0=osl,
                            in1=bet_b[:, oc * OC:(oc + 1) * OC])
                nc.scalar.dma_start(
                    out=out[mi * P:(mi + 1) * P, :], in_=o_t)

            for step in range(RT + LA):
                if PRE <= step < RT:
                    emit_ln(step)
                if step >= LA:
                    emit_mm(step - LA)
    nc.compile()
    return nc


def kernel(**inputs: np.ndarray) -> np.ndarray:
    x = np.asarray(inputs["x"], dtype=np.float32)
    weight = np.asarray(inputs["weight"], dtype=np.float32)
    gamma = np.asarray(inputs["gamma"], dtype=np.float32)
    beta = np.asarray(inputs["beta"], dtype=np.float32)
    ln_weight = np.asarray(inputs["ln_weight"], dtype=np.float32)
    ln_bias = np.asarray(inputs["ln_bias"], dtype=np.float32)
    assert x.shape == (B, R, K), x.shape
    assert weight.shape == (O, K), weight.shape

    triv_lnw = bool(np.all(ln_weight == 1.0))
    triv_lnb = bool(np.all(ln_bias == 0.0))
    triv_gam = bool(np.all(gamma == 1.0))
    triv_bet = bool(np.all(beta == 0.0))
    nc = _build(triv_lnw, triv_lnb, triv_gam, triv_bet)

    in_maps = []
    for c in range(B):
        m = {"x": np.ascontiguousarray(x[c]), "w": weight}
        if not (triv_lnw and triv_lnb and triv_gam and triv_bet):
            m.update({"ln_weight": ln_weight, "ln_bias": ln_bias,
                      "gamma": gamma, "beta": beta})
        in_maps.append(m)

    trace = bool(os.environ.get("BITLIN_TRACE"))
    kwargs = {}
    if trace:
        import concourse.bass_utils as bu
        bu.upload_artifacts = lambda d: d  # keep artifacts local
        tdir = os.environ.get("BITLIN_TRACE_DIR") or None
        kwargs = {"trace": True, "tmpdir": tdir}

    res = run_bass_kernel_spmd(nc, in_maps, core_ids=list(range(B)), **kwargs)
    if trace:
        print(f"HW exec time: {res.exec_time_ns} ns")
    return np.stack([r["out"] for r in res.results], axis=0)
